# revision 1
# baseline (speedup 1.0000x reference)
"""Trainium2 Bass kernel for nn_Bridge_57329223467265 (ragged repeat-interleave).

Reference computation (per batch row b of x [4, 2048, 512]):
    counts = argmax(x @ W + b_vec, -1)            # per-token repeat counts in [0,15]
    csum   = cumsum(counts)                        # inclusive
    out[p] = x[first j with csum[j] > p]  for p < csum[-1], else 0   # p in [0, 30720)

Sharding: 8 cores = 4 batch rows x 2 output halves. Each core receives its
batch row (full x row replicated to its pair), computes logits/argmax/cumsum
on-device, builds the output->source index map C[p] with a scatter + 2-level
prefix-sum trick, and materializes its 15360x512 output slice with one-hot
selection matmuls on the PE (invalid/pad rows fall out as zeros automatically).

kernel(**inputs) takes full unsharded inputs and returns the full [4,30720,512]
output. Everything data-dependent is computed on the NeuronCores.
"""

import os
import numpy as np

from concourse import bass, mybir, bacc, tile
from concourse import bass_utils
from concourse.bass import IndirectOffsetOnAxis
from concourse.masks import make_identity, make_upper_triangular

# Note: walrus --enable-ldw-opt=true fails codegen (visitInstLdweights
# error), so per-matmul weight reloads are unavoidable at the compiler level.

P = 128
S = 2048            # tokens per batch row
D = 512             # feature dim
NCLS = 16           # classes / max repeat-1
NB = 17             # x blocks in SBUF incl one zero pad block
LMAX = S * (NCLS - 1)   # 30720
HALF = LMAX // 2        # 15360 rows per core
NCH = HALF // P         # 120 chunks of 128 output rows

F32 = mybir.dt.float32
F32R = mybir.dt.float32r
F16 = mybir.dt.float16
BF16 = mybir.dt.bfloat16
I32 = mybir.dt.int32
U32 = mybir.dt.uint32
OP = mybir.AluOpType
AX = mybir.AxisListType

# expand matmul dtype variant: "f32" (exact, slow), "f32r" (2-part, exact on HW
# but 4B moving operand runs at ~2cyc/row), "bf16" (2-part hi/lo split,
# ~1e-5 max rel err, 1cyc/row)
VARIANT = "bf16"
DEBUG = False


def build(variant=VARIANT):
    nc = bacc.Bacc("TRN2", target_bir_lowering=False, debug=False, num_devices=8)

    x_dram = nc.dram_tensor("x", [S, D], F32, kind="ExternalInput").ap()
    w_dram = nc.dram_tensor("w", [D, NCLS], F32, kind="ExternalInput").ap()
    b_dram = nc.dram_tensor("bvec", [1, NCLS], F32, kind="ExternalInput").ap()
    p0_dram = nc.dram_tensor("p0", [1, 1], F32, kind="ExternalInput").ap()
    out_dram = nc.dram_tensor("out", [HALF, D], F32, kind="ExternalOutput").ap()
    dbuf = nc.dram_tensor("dbuf", [NCH, P], F32).ap()
    cbuf = nc.dram_tensor("cbuf", [NCH, P], F16).ap()
    dbg = {}
    if DEBUG:
        for nm, shape, dt_ in [
            ("d_csum", [16, P], F32), ("d_delta", [16, P], F32),
            ("d_off", [16, P], F32), ("d_Dsb", [NCH, P], F32),
            ("d_CT", [NCH, P], F32), ("d_Crow", [1, HALF], F16),
            ("d_Blo", [1, NCH], I32), ("d_Bn", [1, NCH], F16),
            ("d_t1", [P, P], F32), ("d_sl", [P, P], F32), ("d_sh", [P, P], F32),
        ]:
            dbg[nm] = nc.dram_tensor(nm, shape, dt_, kind="ExternalOutput").ap()

    with tile.TileContext(nc) as tc:
        _body(tc, x_dram, w_dram, b_dram, p0_dram, out_dram, dbuf, cbuf, variant, dbg)

    nc.compile()
    return nc


def _body(tc, x_dram, w_dram, b_dram, p0_dram, out_dram, dbuf, cbuf, variant, dbg={}):
    nc = tc.nc
    from contextlib import ExitStack

    with ExitStack() as ctx:
        const = ctx.enter_context(tc.tile_pool(name="const", bufs=1))
        work = ctx.enter_context(tc.tile_pool(name="work", bufs=1))
        pipe = ctx.enter_context(tc.tile_pool(name="pipe", bufs=4))

        # ---------------- static tiles ----------------
        ident = const.tile([P, P], F32, tag="ident")
        make_identity(nc, ident[:])
        ustr = const.tile([P, P], F32, tag="ustr")       # 1 where row<col
        make_upper_triangular(nc, ustr[:], 1.0, diag=False)
        ones1 = const.tile([1, P], F32, tag="ones1")
        nc.gpsimd.memset(ones1[:], 1.0)
        ones1h = const.tile([1, P], F16, tag="ones1h")
        nc.gpsimd.memset(ones1h[:], 1.0)
        ones16 = const.tile([16, 1], F32, tag="ones16")
        nc.gpsimd.memset(ones16[:], 1.0)

        GT1 = 4  # chunks per batched T1 matmul / Sel compare
        it_tmp = work.tile([P, GT1 * P], I32, tag="it_tmp")
        nc.gpsimd.iota(it_tmp[:], pattern=[[0, GT1 * P]], base=0, channel_multiplier=1)
        iota_lo = const.tile([P, GT1 * P], F32, tag="iota_lo")   # [s, *] = s
        nc.vector.tensor_copy(iota_lo[:], it_tmp[:])
        iota_hi = const.tile([P, GT1 * P], F32, tag="iota_hi")   # [s, *] = s + 128
        nc.vector.tensor_scalar(iota_hi[:], iota_lo[:], 128.0, None, op0=OP.add)

        it2 = work.tile([16, P], I32, tag="it2")
        nc.gpsimd.iota(it2[:], pattern=[[1, P]], base=1, channel_multiplier=P)
        iotokf = const.tile([16, P], F32, tag="iotokf")    # [m, p] = m*128 + p + 1
        nc.vector.tensor_copy(iotokf[:], it2[:])

        # ---------------- load inputs ----------------
        x_sb = const.tile([P, NB * D], F32, tag="x_sb")
        for m in range(16):
            nc.sync.dma_start(x_sb[:, m * D:(m + 1) * D], x_dram[m * P:(m + 1) * P, :])
        nc.gpsimd.memset(x_sb[:, 16 * D:17 * D], 0.0)      # zero pad block

        w_sb = const.tile([P, 4 * NCLS], F32, tag="w_sb")
        for c in range(4):
            nc.sync.dma_start(w_sb[:, c * NCLS:(c + 1) * NCLS], w_dram[c * P:(c + 1) * P, :])
        b_sb = const.tile([1, NCLS], F32, tag="b_sb")
        nc.sync.dma_start(b_sb[:], b_dram[:])
        p0_sb = const.tile([1, 1], F32, tag="p0_sb")
        nc.sync.dma_start(p0_sb[:], p0_dram[:])

        if variant in ("bf16", "f32r"):
            # two-part hi/lo split: hi = round(x), lo = round(x - hi); one-hot
            # matmuls against both parts accumulate to ~full fp32 precision
            part_dt = BF16 if variant == "bf16" else F32R
            xh = const.tile([P, NB * D], part_dt, tag="xh")
            xl = const.tile([P, NB * D], part_dt, tag="xl")
            for m in range(16):
                sl_ = slice(m * D, (m + 1) * D)
                nc.gpsimd.tensor_copy(xh[:, sl_], x_sb[:, sl_])
                nc.gpsimd.tensor_tensor(xl[:, sl_], x_sb[:, sl_], xh[:, sl_], op=OP.subtract)
            nc.gpsimd.tensor_copy(xh[:, 16 * D:], x_sb[:, 16 * D:])
            nc.gpsimd.tensor_copy(xl[:, 16 * D:], x_sb[:, 16 * D:])

        # zero-fill dbuf
        zt = work.tile([NCH, P], F32, tag="zt")
        nc.gpsimd.memset(zt[:], 0.0)
        nc.sync.dma_start(dbuf[:], zt[:])

        # ---------------- xT + logits + counts ----------------
        with tc.tile_pool(name="psS", bufs=4, space="PSUM") as psS:
            xT = [const.tile([P, S], F32, tag=f"xT{c}", name=f"xT{c}") for c in range(4)]
            for m in range(16):
                for c in range(4):
                    pt = psS.tile([P, P], F32, tag="tr", bufs=2)
                    nc.tensor.transpose(
                        pt[:], x_sb[:, m * D + c * P: m * D + (c + 1) * P], ident[:]
                    )
                    nc.scalar.copy(xT[c][:, m * P:(m + 1) * P], pt[:])

            # logitsT [16, S] with W as the (tiny) stationary operand: 16 big
            # matmuls instead of 80 small ones; bias added per-partition, then
            # transpose 128-token slices back for the free-dim argmax
            bcp = psS.tile([P, 1], F32, tag="sm", bufs=2)
            nc.tensor.transpose(bcp[0:16, 0:1], b_sb[:], ident[0:1, 0:1])
            bcol = work.tile([16, 1], F32, tag="bcol")
            nc.vector.tensor_copy(bcol[:], bcp[0:16, 0:1])

            cntf = const.tile([P, 16], F32, tag="cntf")
            for t4 in range(4):
                plT = psS.tile([16, 4 * P], F32, tag="lgT", bufs=2)
                for c in range(4):
                    nc.tensor.matmul(
                        plT[:], lhsT=w_sb[:, c * NCLS:(c + 1) * NCLS],
                        rhs=xT[c][:, t4 * 4 * P:(t4 + 1) * 4 * P],
                        start=(c == 0), stop=(c == 3),
                    )
                lgT = pipe.tile([16, 4 * P], F32, tag="lgT_sb")
                nc.vector.tensor_scalar(lgT[:], plT[:], bcol[:, 0:1], None, op0=OP.add)
                for u in range(4):
                    m = 4 * t4 + u
                    pb = psS.tile([P, NCLS], F32, tag="lg", bufs=2)
                    nc.tensor.transpose(pb[:, 0:16], lgT[:, u * P:(u + 1) * P],
                                        ident[0:16, 0:16])
                    lg = pipe.tile([P, NCLS], F32, tag="lg_sb")
                    nc.vector.tensor_copy(lg[:], pb[:, 0:16])
                    mx8 = pipe.tile([P, 8], F32, tag="mx8")
                    nc.vector.max(mx8[:], lg[:])
                    mi = pipe.tile([P, 8], U32, tag="mi")
                    nc.vector.max_index(mi[:], mx8[:], lg[:])
                    nc.vector.tensor_copy(cntf[:, m:m + 1], mi[:, 0:1])

            # counts [128,16] -> [16,128]
            ctp = psS.tile([P, P], F32, tag="tr", bufs=2)
            nc.tensor.transpose(ctp[0:16, :], cntf[:], ident[:])
            cT = work.tile([16, P], F32, tag="cT")
            nc.vector.tensor_copy(cT[:], ctp[0:16, :])

            # ---------------- csum ----------------
            csl = work.tile([16, P], F32, tag="csl")
            nc.vector.tensor_tensor_scan(csl[:], cT[:], cT[:], 0.0, op0=OP.add, op1=OP.bypass)
            offp = psS.tile([P, 1], F32, tag="sm", bufs=2)
            nc.tensor.matmul(offp[0:16, :], lhsT=ustr[0:16, 0:16], rhs=csl[:, P - 1:P],
                             start=True, stop=True)
            csum = work.tile([16, P], F32, tag="csum")
            nc.vector.tensor_scalar(csum[:], csl[:], offp[0:16, 0:1], None, op0=OP.add)

            # ---------------- keep / nkeep ----------------
            nxt = work.tile([16, P], F32, tag="nxt")
            nc.vector.tensor_copy(nxt[:, 0:P - 1], csum[:, 1:P])
            # nxt[m,127] = csum[m+1,0] (or -1 for m=15): transpose-shift-transpose
            c0p = psS.tile([P, P], F32, tag="tr", bufs=2)
            nc.tensor.transpose(c0p[0:1, 0:16], csum[:, 0:1], ident[0:16, 0:16])
            c0r = work.tile([1, 16], F32, tag="c0r")
            nc.vector.tensor_copy(c0r[:], c0p[0:1, 0:16])
            c0sh = work.tile([1, 16], F32, tag="c0sh")
            nc.gpsimd.memset(c0sh[:], -1.0)
            nc.vector.tensor_copy(c0sh[0:1, 0:15], c0r[0:1, 1:16])
            c0cp = psS.tile([P, 1], F32, tag="sm", bufs=2)
            nc.tensor.transpose(c0cp[0:16, 0:1], c0sh[:], ident[0:1, 0:1])
            nc.vector.tensor_copy(nxt[:, P - 1:P], c0cp[0:16, 0:1])
            keep = work.tile([16, P], F32, tag="keep")
            nc.vector.tensor_tensor(keep[:], csum[:], nxt[:], op=OP.not_equal)
            nkeep = work.tile([16, P], F32, tag="nkeep")
            nc.vector.tensor_tensor(nkeep[:], csum[:], nxt[:], op=OP.is_equal)

            # ---------------- delta = j+1 - prev_kept ----------------
            E = work.tile([16, P], F32, tag="E")
            nc.vector.tensor_tensor(E[:], keep[:], iotokf[:], op=OP.mult)
            Esh = work.tile([16, P], F32, tag="Esh")
            nc.vector.tensor_copy(Esh[:, 1:P], E[:, 0:P - 1])
            # Esh[m,0] = E[m-1,127] (0 for m=0): transpose-shift-transpose
            e1p = psS.tile([P, P], F32, tag="tr", bufs=2)
            nc.tensor.transpose(e1p[0:1, 0:16], E[:, P - 1:P], ident[0:16, 0:16])
            e1r = work.tile([1, 16], F32, tag="e1r")
            nc.vector.tensor_copy(e1r[:], e1p[0:1, 0:16])
            e1sh = work.tile([1, 16], F32, tag="e1sh")
            nc.gpsimd.memset(e1sh[:], 0.0)
            nc.vector.tensor_copy(e1sh[0:1, 1:16], e1r[0:1, 0:15])
            e1cp = psS.tile([P, 1], F32, tag="sm", bufs=2)
            nc.tensor.transpose(e1cp[0:16, 0:1], e1sh[:], ident[0:1, 0:1])
            nc.vector.tensor_copy(Esh[:, 0:1], e1cp[0:16, 0:1])
            Mloc = work.tile([16, P], F32, tag="Mloc")
            nc.vector.tensor_tensor_scan(Mloc[:], Esh[:], Esh[:], 0.0, op0=OP.max, op1=OP.bypass)
            bm = work.tile([16, 1], F32, tag="bm")
            nc.vector.tensor_reduce(bm[:], E[:], axis=AX.X, op=OP.max)
            bmp = psS.tile([P, P], F32, tag="tr", bufs=2)
            nc.tensor.transpose(bmp[0:1, 0:16], bm[:], ident[0:16, 0:16])
            bmr = work.tile([1, 16], F32, tag="bmr")
            nc.vector.tensor_copy(bmr[:], bmp[0:1, 0:16])
            bmsh = work.tile([1, 16], F32, tag="bmsh")
            nc.vector.tensor_copy(bmsh[:, 1:16], bmr[:, 0:15])
            nc.gpsimd.memset(bmsh[0:1, 0:1], 0.0)
            bms = work.tile([1, 16], F32, tag="bms")
            nc.vector.tensor_tensor_scan(bms[:], bmsh[:], bmsh[:], 0.0, op0=OP.max, op1=OP.bypass)
            crp = psS.tile([P, 1], F32, tag="sm", bufs=2)
            nc.tensor.transpose(crp[0:16, 0:1], bms[:], ident[0:1, 0:1])
            Pex = work.tile([16, P], F32, tag="Pex")
            nc.vector.tensor_scalar(Pex[:], Mloc[:], crp[0:16, 0:1], None, op0=OP.max)
            delta = work.tile([16, P], F32, tag="delta")
            nc.vector.tensor_tensor(delta[:], iotokf[:], Pex[:], op=OP.subtract)
            dkept = work.tile([16, P], F32, tag="dkept")
            nc.vector.tensor_tensor(dkept[:], delta[:], keep[:], op=OP.mult)

            # ---------------- per-core half: base + scatter offsets ----------------
            # broadcast p0 to [16,1]
            p0p = psS.tile([P, 1], F32, tag="sm", bufs=2)
            nc.tensor.matmul(p0p[0:16, :], lhsT=ones1[0:1, 0:16], rhs=p0_sb[:],
                             start=True, stop=True)
            p0b = work.tile([16, 1], F32, tag="p0b")
            nc.vector.tensor_copy(p0b[:], p0p[0:16, :])

            qf = work.tile([16, P], F32, tag="qf")
            nc.vector.tensor_scalar(qf[:], csum[:], p0b[:, 0:1], None, op0=OP.subtract)
            m1 = work.tile([16, P], F32, tag="m1")
            nc.vector.tensor_scalar(m1[:], qf[:], 0.0, None, op0=OP.is_lt)
            m2 = work.tile([16, P], F32, tag="m2")
            nc.vector.tensor_scalar(m2[:], qf[:], float(HALF), None, op0=OP.is_ge)
            mm = work.tile([16, P], F32, tag="mm")
            nc.vector.tensor_tensor(mm[:], m1[:], m2[:], op=OP.add)
            mm2 = work.tile([16, P], F32, tag="mm2")
            nc.vector.tensor_tensor(mm2[:], mm[:], nkeep[:], op=OP.add)
            offf = work.tile([16, P], F32, tag="offf")
            nc.vector.scalar_tensor_tensor(offf[:], in0=mm2[:], scalar=1.0e7, in1=qf[:],
                                           op0=OP.mult, op1=OP.add)

            # base = sum(dkept * (csum < p0))
            bsc = work.tile([16, P], F32, tag="bsc")
            pp = work.tile([16, 1], F32, tag="pp")
            nc.vector.scalar_tensor_tensor(bsc[:], in0=csum[:], scalar=p0b[:, 0:1],
                                           in1=dkept[:], op0=OP.is_lt, op1=OP.mult,
                                           accum_out=pp[:])
            basep = psS.tile([P, 1], F32, tag="sm", bufs=2)
            nc.tensor.matmul(basep[0:1, 0:1], lhsT=pp[:], rhs=ones16[:], start=True, stop=True)
            base_sb = work.tile([1, 1], F32, tag="base_sb")
            nc.vector.tensor_copy(base_sb[:], basep[0:1, 0:1])

            # transpose delta [16,128]->[128,16], offf likewise
            dtp = psS.tile([P, P], F32, tag="tr", bufs=2)
            nc.tensor.transpose(dtp[:, 0:16], delta[:], ident[0:16, 0:16])
            dT = work.tile([P, 16], F32, tag="dT")
            nc.vector.tensor_copy(dT[:], dtp[:, 0:16])
            otp = psS.tile([P, P], F32, tag="tr", bufs=2)
            nc.tensor.transpose(otp[:, 0:16], offf[:], ident[0:16, 0:16])
            oTi = work.tile([P, 16], I32, tag="oTi")
            nc.vector.tensor_copy(oTi[:], otp[:, 0:16])

            # ---------------- scatter deltas into dbuf ----------------
            dbuf_flat = dbuf.rearrange("k (p one) -> (k p) one", one=1)
            for c in range(16):
                nc.gpsimd.indirect_dma_start(
                    out=dbuf_flat,
                    out_offset=IndirectOffsetOnAxis(ap=oTi[:, c:c + 1], axis=0),
                    in_=dT[:, c:c + 1],
                    in_offset=None,
                    bounds_check=HALF - 1,
                    oob_is_err=False,
                )

            # ---------------- readback + 2-level scan -> C ----------------
            D_sb = work.tile([NCH, P], F32, tag="D_sb")
            nc.sync.dma_start(D_sb[:], dbuf[:])
            S1 = work.tile([NCH, P], F32, tag="S1")
            nc.vector.tensor_tensor_scan(S1[:], D_sb[:], D_sb[:], 0.0, op0=OP.add, op1=OP.bypass)
            carp = psS.tile([P, 1], F32, tag="sm", bufs=2)
            nc.tensor.matmul(carp[:], lhsT=ustr[0:NCH, :], rhs=S1[:, P - 1:P],
                             start=True, stop=False)
            nc.tensor.matmul(carp[:], lhsT=ones1[:], rhs=base_sb[:], start=False, stop=True)
            C_T = work.tile([NCH, P], F32, tag="C_T")
            nc.vector.tensor_scalar(C_T[:], S1[:], carp[0:NCH, 0:1], None, op0=OP.add)
            iv = work.tile([NCH, P], F32, tag="iv")
            nc.vector.tensor_scalar(iv[:], C_T[:], float(S), None, op0=OP.is_ge)
            C_adj = work.tile([NCH, P], F32, tag="C_adj")
            nc.vector.scalar_tensor_tensor(C_adj[:], in0=iv[:], scalar=30000.0, in1=C_T[:],
                                           op0=OP.mult, op1=OP.add)

            # ---------------- window base per chunk ----------------
            # Bcol[k] = clip(C[k*128] >> 7, 0, 15)  (as [NCH,1] column)
            t0ci = work.tile([NCH, 1], I32, tag="t0ci")
            nc.vector.tensor_copy(t0ci[:], C_T[:, 0:1])
            bc1 = work.tile([NCH, 1], I32, tag="bc1")
            nc.vector.tensor_scalar(bc1[:], t0ci[:], 7, None, op0=OP.arith_shift_right)
            bc2 = work.tile([NCH, 1], I32, tag="bc2")
            nc.vector.tensor_scalar(bc2[:], bc1[:], 15, None, op0=OP.min)
            bcf = work.tile([NCH, 1], F32, tag="bcf")
            nc.vector.tensor_copy(bcf[:], bc2[:])
            b128c = work.tile([NCH, 1], F32, tag="b128c")
            nc.vector.tensor_scalar(b128c[:], bcf[:], 128.0, None, op0=OP.mult)

            # window-relative index row: C_rel = C_adj - 128*B (invalid stays big)
            C_rel = work.tile([NCH, P], F16, tag="C_rel")
            nc.vector.tensor_scalar(C_rel[:], C_adj[:], b128c[:, 0:1], None, op0=OP.subtract)
            # flatten [120,128] -> single-partition row so chunk rows are
            # free-dim slices at partition 0 (engine SBUF APs must start at
            # partition 0/32/64/96); bounce through DRAM to cross partitions
            C_row = const.tile([1, HALF], F16, tag="C_row")
            nc.sync.dma_start(cbuf[:], C_rel[:])
            nc.sync.dma_start(
                C_row[0:1, :], cbuf.rearrange("(one k) p -> one (k p)", one=1)
            )

            # Blo/Bhi rows (byte.. element offsets into x blocks) via transpose
            t0p = psS.tile([P, P], F32, tag="tr", bufs=2)
            nc.tensor.transpose(t0p[0:1, 0:NCH], bcf[:], ident[0:NCH, 0:NCH])
            t0r = work.tile([1, NCH], F32, tag="t0r")
            nc.vector.tensor_copy(t0r[:], t0p[0:1, 0:NCH])
            t0i = work.tile([1, NCH], I32, tag="t0i")
            nc.vector.tensor_copy(t0i[:], t0r[:])
            Blo = const.tile([1, NCH], I32, tag="Blo")
            nc.vector.tensor_scalar(Blo[:], t0i[:], 9, None, op0=OP.logical_shift_left)

        if dbg:
            nc.sync.dma_start(dbg["d_csum"], csum[:])
            nc.sync.dma_start(dbg["d_delta"], delta[:])
            nc.sync.dma_start(dbg["d_off"], offf[:])
            nc.sync.dma_start(dbg["d_Dsb"], D_sb[:])
            nc.sync.dma_start(dbg["d_CT"], C_T[:])
            nc.sync.dma_start(dbg["d_Crow"], C_row[:])
            nc.sync.dma_start(dbg["d_Blo"], Blo[:])

        # ---------------- main expand loop ----------------
        # one-hot Sel matrices hold only 0.0/1.0 (exact in fp8e4), and fp8
        # stationaries halve the LDWEIGHTS stream vs bf16
        seldt = {"f32": F32, "f32r": F32R, "bf16": mybir.dt.float8e4}[variant]
        with (
            tc.tile_pool(name="psT1", bufs=2, space="PSUM") as psT1,
            tc.tile_pool(name="psO", bufs=6, space="PSUM") as psO,
            tc.tile_pool(name="selp", bufs=4) as selp,
            tc.tile_pool(name="outp", bufs=6) as outp,
        ):
            GRP = 16  # chunks per batched register load
            for k in range(NCH):
                if k % GRP == 0:
                    n = min(GRP, NCH - k)
                    _, vals = nc.values_load_multi_w_load_instructions(
                        Blo[0:1, k:k + n],
                        engines={mybir.EngineType.PE},
                        min_val=0, max_val=15 * D,
                        skip_runtime_bounds_check=True,
                    )
                vlo = vals[k % GRP]
                vhi = vlo + D
                if k % GT1 == 0:
                    t1 = psT1.tile([P, GT1 * P], F32, tag="T1")
                    nc.tensor.matmul(t1[:], lhsT=ones1h[:],
                                     rhs=C_row[0:1, k * P:(k + GT1) * P],
                                     start=True, stop=True, tile_position=(0, 0))
                    sl4 = selp.tile([P, GT1 * P], seldt, tag="sl")
                    nc.vector.tensor_tensor(sl4[:], iota_lo[:], t1[:], op=OP.is_equal)
                    sh4 = selp.tile([P, GT1 * P], seldt, tag="sh")
                    nc.vector.tensor_tensor(sh4[:], iota_hi[:], t1[:], op=OP.is_equal)
                j = (k % GT1) * P
                sl = sl4[:, j:j + P]
                sh = sh4[:, j:j + P]
                if dbg and k == 0:
                    t1c = selp.tile([P, P], F32, tag="t1c")
                    nc.vector.tensor_copy(t1c[:], t1[:, 0:P])
                    nc.sync.dma_start(dbg["d_t1"], t1c[:])
                    nc.sync.dma_start(dbg["d_sl"], sl.bitcast(F32))
                    nc.sync.dma_start(dbg["d_sh"], sh.bitcast(F32))

                po = psO.tile([P, D], F32, tag="po")
                if variant == "f32":
                    nc.tensor.matmul(po[:], lhsT=sl, rhs=x_sb[:, bass.ds(vlo, D)],
                                     start=True, stop=False)
                    nc.tensor.matmul(po[:], lhsT=sh, rhs=x_sb[:, bass.ds(vhi, D)],
                                     start=False, stop=True)
                else:  # two-part hi/lo (bf16 or f32r), same-lhsT pairs adjacent
                    nc.tensor.matmul(po[:], lhsT=sl, rhs=xh[:, bass.ds(vlo, D)],
                                     start=True, stop=False)
                    nc.tensor.matmul(po[:], lhsT=sl, rhs=xl[:, bass.ds(vlo, D)],
                                     start=False, stop=False)
                    nc.tensor.matmul(po[:], lhsT=sh, rhs=xh[:, bass.ds(vhi, D)],
                                     start=False, stop=False)
                    nc.tensor.matmul(po[:], lhsT=sh, rhs=xl[:, bass.ds(vhi, D)],
                                     start=False, stop=True)

                ob = outp.tile([P, D], F32, tag="ob")
                if k % 2 == 0:
                    nc.vector.tensor_copy(ob[:], po[:])
                else:
                    nc.scalar.copy(ob[:], po[:])
                nc.sync.dma_start(out_dram[k * P:(k + 1) * P, :], ob[:])


# ---------------------------------------------------------------------------
_BUILT = {}


def _get_built(variant=VARIANT):
    if variant not in _BUILT:
        _BUILT[variant] = build(variant)
    return _BUILT[variant]


def make_in_maps(x, W, b):
    in_maps = []
    for core in range(8):
        bi, h = core // 2, core % 2
        in_maps.append({
            "x": np.ascontiguousarray(x[bi]).astype(np.float32),
            "w": np.ascontiguousarray(W).astype(np.float32),
            "bvec": np.ascontiguousarray(b).reshape(1, NCLS).astype(np.float32),
            "p0": np.array([[float(h * HALF)]], dtype=np.float32),
        })
    return in_maps


def assemble(outs):
    return np.stack(
        [np.concatenate([outs[2 * b], outs[2 * b + 1]], axis=0) for b in range(4)]
    )


def kernel(x, W, b):
    nc = _get_built()
    res = bass_utils.run_bass_kernel_spmd(nc, make_in_maps(x, W, b),
                                          core_ids=list(range(8)))
    return assemble([res.results[c]["out"] for c in range(8)])


if __name__ == "__main__":
    nc = build()
    print("build OK:", len(nc.m.functions[0].instructions) if hasattr(nc.m.functions[0], "instructions") else "n/a")

